# revision 1
# baseline (speedup 1.0000x reference)
"""MultiHeadAttention Trainium2 kernel (8 NeuronCores).

Sharding: 8 cores = 4 batches x 2 head-groups (8 heads each).
Core g: batch b = g//2, head-group hg = g%2 (heads hg*8 .. hg*8+7).

Device program (identical on all cores, SPMD):
  inputs (per core): xq/xk/xv = x[b].T  [1024, 2048] (f32r),
    wq/wk/wv = w[:, hg*512:(hg+1)*512]  [1024, 512] (f32r),
    wo = w_o[hg*512:(hg+1)*512, :]      [512, 1024] (f32r),
    bq = b_q slice reshaped [4, 128] (f32)
  output: yt [1024, 2048] = (partial out).T for this batch/head-group,
    unnormalized by biases (host adds b_v @ w_o + b_o once per batch).

Math identities used (exact in real arithmetic):
  softmax((Q+bq)(K+bk)^T) == softmax((Q+bq) K^T)   [k-constant terms cancel]
  attn @ (V + bv) @ Wo + bo == attn @ V @ Wo + (bv @ Wo + bo)  [rows sum to 1]
  exp without max-subtraction is safe: scores ~ N(0,1), max < ~6.

Layouts: QhT/KhT [128 = head-pair d, 2048 seq] per pair; Vh [128 k-chunk,
8 heads x (64 dv + ones-col)]; ones-col makes the AV matmul also produce
Z = sum_k exp(s) at psum row 64.
"""
import numpy as np

B, S, D = 4, 2048, 1024
HPC, PAIRS, QB, KC, CC = 8, 4, 4, 16, 8  # heads/core, pairs, 512-q-blocks, 128-k-chunks, 128-c-chunks
N = 512

_CACHE = {}


def _build():
    from concourse import bacc
    import concourse.mybir as mybir
    import concourse.tile as tile

    F32 = mybir.dt.float32
    F32R = mybir.dt.float32r
    AF = mybir.ActivationFunctionType

    nc = bacc.Bacc()
    xq_d = nc.declare_dram_parameter("xq", [D, S], F32R, isOutput=False)
    xk_d = nc.declare_dram_parameter("xk", [D, S], F32R, isOutput=False)
    xv_d = nc.declare_dram_parameter("xv", [D, S], F32R, isOutput=False)
    wq_d = nc.declare_dram_parameter("wq", [D, N], F32R, isOutput=False)
    wk_d = nc.declare_dram_parameter("wk", [D, N], F32R, isOutput=False)
    wv_d = nc.declare_dram_parameter("wv", [D, N], F32R, isOutput=False)
    wo_d = nc.declare_dram_parameter("wo", [N, D], F32R, isOutput=False)
    bq_d = nc.declare_dram_parameter("bq", [PAIRS, 128], F32, isOutput=False)
    ones_d = nc.declare_dram_parameter("ones", [128, HPC], F32R, isOutput=False)
    yt_d = nc.dram_tensor("yt", [D, S], F32, kind="ExternalOutput")

    with tile.TileContext(nc) as tc:
        with (
            tc.tile_pool(name="per", bufs=1) as per,
            tc.tile_pool(name="wp", bufs=1) as wp,
            tc.tile_pool(name="xs", bufs=1) as xsp,
            tc.tile_pool(name="ep", bufs=1) as epp,
            tc.tile_pool(name="msc", bufs=1) as msc,
        ):
            # ---- persistent tiles ----
            kh = [per.tile([128, S], F32R, name=f"kh{p}", tag="kh", bufs=PAIRS)
                  for p in range(PAIRS)]
            qh = [per.tile([128, S], F32R, name=f"qh{p}", tag="qh", bufs=PAIRS)
                  for p in range(PAIRS)]
            vs = [per.tile([128, HPC * 65], F32R, name=f"vs{t}", tag="vs", bufs=KC)
                  for t in range(KC)]
            bqt = per.tile([128, PAIRS], F32, name="bqt", tag="bqt", bufs=1)
            for p in range(PAIRS):
                nc.sync.dma_start(out=bqt[:, p:p + 1], in_=bq_d[p, :])

            # ---- weights (tag-shared slots; wo reuses wk's slot later) ----
            wk_s = wp.tile([128, CC, N], F32R, name="wk_s", tag="w2", bufs=2)
            wv_s = wp.tile([128, CC, N], F32R, name="wv_s", tag="w2", bufs=2)
            wq_s = wp.tile([128, CC, N], F32R, name="wq_s", tag="w2", bufs=2)
            nc.sync.dma_start(out=wk_s, in_=wk_d.rearrange("(c p) n -> p c n", p=128))
            nc.scalar.dma_start(out=wv_s, in_=wv_d.rearrange("(c p) n -> p c n", p=128))
            nc.sync.dma_start(out=wq_s, in_=wq_d.rearrange("(c p) n -> p c n", p=128))

            # ================= phase 1: projections =================
            with tc.tile_pool(name="pp", bufs=1, space="PSUM") as pp:
                # K-proj and Q-proj: out [pair-d 128, seq] per (pair, qblock)
                for w_s, dst, x_d, biased in ((wk_s, kh, xk_d, False),
                                              (wq_s, qh, xq_d, True)):
                    for j in range(QB):
                        xt = [xsp.tile([128, N], F32R, name=f"xt{c}", tag="xs", bufs=8)
                              for c in range(CC)]
                        for c in range(CC):
                            eng = nc.sync if c % 2 == 0 else nc.scalar
                            eng.dma_start(
                                out=xt[c],
                                in_=x_d[128 * c:128 * (c + 1), N * j:N * (j + 1)])
                        for p in range(PAIRS):
                            psU = pp.tile([128, N], F32, name="psU", tag="proj", bufs=8)
                            psL = pp.tile([128, N], F32, name="psL", tag="proj", bufs=8)
                            for c in range(CC):
                                nc.tensor.matmul(psU, w_s[0:64, c, 128 * p:128 * (p + 1)],
                                                 xt[c][0:64, :], start=(c == 0),
                                                 stop=(c == CC - 1), tile_position=(0, 0))
                                nc.tensor.matmul(psL, w_s[64:128, c, 128 * p:128 * (p + 1)],
                                                 xt[c][64:128, :], start=(c == 0),
                                                 stop=(c == CC - 1), tile_position=(64, 0))
                            ptmp = msc.tile([128, N], F32, name="ptmp", tag="ptmp", bufs=2)
                            if biased:
                                nc.vector.tensor_scalar_add(ptmp, psU, bqt[:, p:p + 1])
                            else:
                                nc.vector.tensor_copy(ptmp, psU)
                            nc.vector.tensor_add(dst[p][:, N * j:N * (j + 1)], ptmp, psL)
                # V-proj: out [k 128, dv 512] per k-tile; strided into vs + ones col
                for q4 in range(QB):
                    xt = [xsp.tile([128, N], F32R, name=f"xvt{c}", tag="xs", bufs=8)
                          for c in range(CC)]
                    for c in range(CC):
                        eng = nc.sync if c % 2 == 0 else nc.scalar
                        eng.dma_start(
                            out=xt[c],
                            in_=xv_d[128 * c:128 * (c + 1), N * q4:N * (q4 + 1)])
                    for t2 in range(4):
                        t = 4 * q4 + t2
                        psU = pp.tile([128, N], F32, name="psvU", tag="proj", bufs=8)
                        psL = pp.tile([128, N], F32, name="psvL", tag="proj", bufs=8)
                        for c in range(CC):
                            nc.tensor.matmul(psU, xt[c][0:64, 128 * t2:128 * (t2 + 1)],
                                             wv_s[0:64, c, :], start=(c == 0),
                                             stop=(c == CC - 1), tile_position=(0, 0))
                            nc.tensor.matmul(psL, xt[c][64:128, 128 * t2:128 * (t2 + 1)],
                                             wv_s[64:128, c, :], start=(c == 0),
                                             stop=(c == CC - 1), tile_position=(64, 0))
                        v3 = vs[t].rearrange("p (h e) -> p h e", e=65)
                        nc.sync.dma_start(out=v3[:, :, 64:65], in_=ones_d[:, :])
                        ptmp = msc.tile([128, N], F32, name="ptmpv", tag="ptmp", bufs=2)
                        nc.vector.tensor_copy(ptmp, psU)
                        nc.vector.tensor_add(
                            v3[:, :, 0:64], ptmp.rearrange("p (h e) -> p h e", e=64),
                            psL.rearrange("p (h e) -> p h e", e=64))

            # wo loaded into a freed w2 slot
            wo_s = wp.tile([128, PAIRS, D], F32R, name="wo_s", tag="w2", bufs=2)
            nc.scalar.dma_start(out=wo_s, in_=wo_d.rearrange("(i p) n -> p i n", p=128))

            # ================= phase 2: attention + out-proj =================
            # Flattened (j, p, g) stream: AV lags scores/exp by 2 groups so PE
            # never waits on ACT; po psums released early via raw copies; the
            # out-projection of qblock j is interleaved into qblock j+1.
            with (
                tc.tile_pool(name="sc", bufs=1, space="PSUM") as scp,
                tc.tile_pool(name="avp", bufs=1, space="PSUM") as avp,
            ):
                NG = KC // 2
                stream = [(j, p, g) for j in range(QB) for p in range(PAIRS)
                          for g in range(NG)]
                ctx = {}     # (j, p) -> dict(po0, po1, eA[g], eB[g])
                ots = {}     # j -> [ot tiles]
                oproj_pending = []

                def emit_scores_exp(j, p, g):
                    if g == 0:
                        ctx[(j, p)] = {
                            "po0a": avp.tile([65, N], F32, name="po0a", tag="av", bufs=4),
                            "po0b": avp.tile([65, N], F32, name="po0b", tag="av", bufs=4),
                            "po1a": avp.tile([65, N], F32, name="po1a", tag="av", bufs=4),
                            "po1b": avp.tile([65, N], F32, name="po1b", tag="av", bufs=4),
                            "eA": [None] * NG, "eB": [None] * NG,
                        }
                    st_ = ctx[(j, p)]
                    sA = scp.tile([128, 2 * N], F32, name="sA", tag="sc", bufs=2)
                    sB = scp.tile([128, 2 * N], F32, name="sB", tag="sc", bufs=2)
                    for ci in range(2):
                        c = 2 * g + ci
                        nc.tensor.matmul(
                            sA[:, N * ci:N * (ci + 1)],
                            kh[p][0:64, 128 * c:128 * (c + 1)],
                            qh[p][0:64, N * j:N * (j + 1)],
                            start=True, stop=True, tile_position=(0, 0))
                        nc.tensor.matmul(
                            sB[:, N * ci:N * (ci + 1)],
                            kh[p][64:128, 128 * c:128 * (c + 1)],
                            qh[p][64:128, N * j:N * (j + 1)],
                            start=True, stop=True, tile_position=(64, 0))
                    eA = epp.tile([128, 2 * N], F32R, name="eA", tag="ep", bufs=4)
                    eB = epp.tile([128, 2 * N], F32R, name="eB", tag="ep", bufs=4)
                    nc.scalar.activation(eA, sA, AF.Exp, scale=0.125)
                    nc.scalar.activation(eB, sB, AF.Exp, scale=0.125)
                    st_["eA"][g], st_["eB"][g] = eA, eB

                def emit_av(j, p, g):
                    st_ = ctx[(j, p)]
                    h0, h1 = 2 * p, 2 * p + 1
                    for ci in range(2):
                        c = 2 * g + ci
                        ss, se = (c == 0), (c == KC - 1)
                        eAg = st_["eA"][g][:, N * ci:N * (ci + 1)]
                        eBg = st_["eB"][g][:, N * ci:N * (ci + 1)]
                        nc.tensor.matmul(
                            st_["po0a"], vs[c][0:64, 65 * h0:65 * h0 + 65],
                            eAg[0:64, :], start=ss, stop=se, tile_position=(0, 0))
                        nc.tensor.matmul(
                            st_["po0b"], vs[c][64:128, 65 * h0:65 * h0 + 65],
                            eAg[64:128, :], start=ss, stop=se, tile_position=(64, 0))
                        nc.tensor.matmul(
                            st_["po1a"], vs[c][0:64, 65 * h1:65 * h1 + 65],
                            eBg[0:64, :], start=ss, stop=se, tile_position=(0, 0))
                        nc.tensor.matmul(
                            st_["po1b"], vs[c][64:128, 65 * h1:65 * h1 + 65],
                            eBg[64:128, :], start=ss, stop=se, tile_position=(64, 0))

                def emit_norm(j, p):
                    st_ = ctx.pop((j, p))
                    if j not in ots:
                        ots[j] = [epp.tile([128, N], F32R, name=f"ot{q}", tag="ot",
                                           bufs=4) for q in range(PAIRS)]
                    ot = ots[j]
                    raw = msc.tile([65, 2 * N], F32, name="raw", tag="raw", bufs=1)
                    rawc = msc.tile([65, 2 * N], F32, name="rawc", tag="rawc", bufs=1)
                    nc.vector.tensor_copy(rawc[:, 0:N], st_["po0a"])
                    nc.vector.tensor_copy(rawc[:, N:2 * N], st_["po1a"])
                    nc.vector.tensor_add(raw[:, 0:N], rawc[:, 0:N], st_["po0b"])
                    nc.vector.tensor_add(raw[:, N:2 * N], rawc[:, N:2 * N], st_["po1b"])
                    zstA = msc.tile([1, N], F32, name="zstA", tag="zst", bufs=2)
                    zstB = msc.tile([1, N], F32, name="zstB", tag="zst", bufs=2)
                    nc.gpsimd.dma_start(out=zstA, in_=raw[64:65, 0:N])
                    nc.gpsimd.dma_start(out=zstB, in_=raw[64:65, N:2 * N])
                    zbc = msc.tile([128, N], F32, name="zbc", tag="zbc", bufs=1)
                    zb2 = msc.tile([128, N], F32, name="zb2", tag="zb2", bufs=1)
                    nc.gpsimd.partition_broadcast(zbc, zstA[0:1, :])
                    nc.gpsimd.partition_broadcast(zb2, zstB[0:1, :])
                    nc.vector.tensor_copy(zbc[64:128, :], zb2[64:128, :])
                    rbc = msc.tile([128, N], F32, name="rbc", tag="rbc", bufs=1)
                    nc.vector.reciprocal(rbc, zbc)
                    rbcB = msc.tile([64, N], F32, name="rbcB", tag="rbcB", bufs=1)
                    nc.gpsimd.dma_start(out=rbcB, in_=rbc[64:128, :])
                    nc.vector.tensor_mul(ot[p][0:64, :], raw[0:64, 0:N], rbc[0:64, :])
                    tmp1 = msc.tile([64, N], F32R, name="tmp1", tag="tmp1", bufs=1)
                    nc.vector.tensor_mul(tmp1, raw[0:64, N:2 * N], rbcB)
                    nc.gpsimd.dma_start(out=ot[p][64:128, :], in_=tmp1)
                    if p == PAIRS - 1:
                        for e in range(8):
                            oproj_pending.append((j, e))

                def emit_oproj_chunk():
                    j2, e = oproj_pending.pop(0)
                    ot = ots[j2]
                    py = scp.tile([128, N], F32, name="py", tag="sc", bufs=2)
                    for p2 in range(PAIRS):
                        nc.tensor.matmul(py, wo_s[:, p2, 128 * e:128 * (e + 1)],
                                         ot[p2], start=(p2 == 0), stop=(p2 == PAIRS - 1))
                    ys = msc.tile([128, N], F32, name="ys", tag="ys", bufs=2)
                    nc.vector.tensor_copy(ys, py)
                    nc.sync.dma_start(
                        out=yt_d[128 * e:128 * (e + 1), N * j2:N * (j2 + 1)], in_=ys)
                    if e == 7:
                        del ots[j2]

                LAG = 2
                for idx, (j, p, g) in enumerate(stream):
                    emit_scores_exp(j, p, g)
                    if idx >= LAG:
                        j2, p2, g2 = stream[idx - LAG]
                        emit_av(j2, p2, g2)
                        if g2 == NG - 1:
                            emit_norm(j2, p2)
                    if oproj_pending:
                        emit_oproj_chunk()
                for k in range(LAG):
                    j2, p2, g2 = stream[len(stream) - LAG + k]
                    emit_av(j2, p2, g2)
                    if g2 == NG - 1:
                        emit_norm(j2, p2)
                while oproj_pending:
                    emit_oproj_chunk()

    nc.compile()
    return nc


def _get_nc():
    if "nc" not in _CACHE:
        _CACHE["nc"] = _build()
    return _CACHE["nc"]


def kernel(q, k, v, w_q, b_q, w_k, b_k, w_v, b_v, w_o, b_o):
    import sys, types

    # NTFF/upload shims are only needed for tracing; plain run needs neither.
    nc = _get_nc()
    from concourse.bass_utils import run_bass_kernel_spmd

    q = np.asarray(q, dtype=np.float32)
    k = np.asarray(k, dtype=np.float32)
    v = np.asarray(v, dtype=np.float32)
    w_q = np.asarray(w_q, dtype=np.float32)
    w_k = np.asarray(w_k, dtype=np.float32)
    w_v = np.asarray(w_v, dtype=np.float32)
    w_o = np.asarray(w_o, dtype=np.float32)
    b_q = np.asarray(b_q, dtype=np.float32)
    b_v = np.asarray(b_v, dtype=np.float32)
    b_o = np.asarray(b_o, dtype=np.float32)

    xT = {}
    for b in range(B):
        xT[("q", b)] = np.ascontiguousarray(q[b].T)
        xT[("k", b)] = np.ascontiguousarray(k[b].T)
        xT[("v", b)] = np.ascontiguousarray(v[b].T)
    in_maps = []
    for g in range(8):
        b, hg = g // 2, g % 2
        sl = slice(hg * 512, (hg + 1) * 512)
        in_maps.append({
            "xq": xT[("q", b)], "xk": xT[("k", b)], "xv": xT[("v", b)],
            "wq": np.ascontiguousarray(w_q[:, sl]),
            "wk": np.ascontiguousarray(w_k[:, sl]),
            "wv": np.ascontiguousarray(w_v[:, sl]),
            "wo": np.ascontiguousarray(w_o[sl, :]),
            "bq": np.ascontiguousarray(b_q[sl].reshape(PAIRS, 128)),
            "ones": np.ones((128, HPC), dtype=np.float32),
        })

    res = run_bass_kernel_spmd(nc, in_maps, list(range(8)), trace=False)
    outs = [r["yt"] for r in res.results]

    corr = b_v @ w_o + b_o  # [1024]
    y = np.empty((B, S, D), dtype=np.float32)
    for b in range(B):
        y[b] = outs[2 * b].T + outs[2 * b + 1].T + corr
    return y



# revision 2
# speedup vs baseline: 1.2243x; 1.2243x over previous
"""MultiHeadAttention Trainium2 kernel (8 NeuronCores).

Sharding: 8 cores = 4 batches x 2 head-groups (8 heads each).
Core g: batch b = g//2, head-group hg = g%2 (heads hg*8 .. hg*8+7).

Device program (identical on all cores, SPMD):
  inputs (per core): xq/xk = x[b].T  [1024, 2048] (f32r), xv bf16,
    wq/wk = w[:, hg*512:(hg+1)*512]  [1024, 512] (f32r), wv bf16,
    wo = w_o[hg*512:(hg+1)*512, :]   [512, 1024] (bf16),
    bq = b_q slice reshaped [4, 128] (f32)
  output: yt [1024, 2048] = (partial out).T for this batch/head-group,
    unnormalized by biases (host adds b_v @ w_o + b_o once per batch).

Math identities used (exact in real arithmetic):
  softmax((Q+bq)(K+bk)^T) == softmax((Q+bq) K^T)   [k-constant terms cancel]
  attn @ (V + bv) @ Wo + bo == attn @ V @ Wo + (bv @ Wo + bo)  [rows sum to 1]
  exp without max-subtraction is safe: scores ~ N(0,1), max < ~6.

Dtypes: Q/K path f32r (score exponents are precision-sensitive); V/exp/AV/
out-proj bf16 (post-softmax linear path, errors average out); psum f32.

Matmuls use full K=128 contraction everywhere except QK^T scores (d=64
per head, two heads packed as row-tile pairs at (0,0)/(64,0)).

Layouts: QhT/KhT [128 = head-pair d, 2048 seq] per pair; Vh [128 k-chunk,
8 heads x (64 dv + ones-col)] bf16; ones-col makes the AV matmul also
produce Z = sum_k exp(s) at psum row 64.
"""
import numpy as np

B, S, D = 4, 2048, 1024
HPC, PAIRS, QB, KC, CC = 8, 4, 4, 16, 8  # heads/core, pairs, 512-q-blocks, 128-k-chunks, 128-c-chunks
N = 512

_CACHE = {}


def _build():
    from concourse import bacc
    import concourse.mybir as mybir
    import concourse.tile as tile

    F32 = mybir.dt.float32
    F32R = mybir.dt.float32r
    BF16 = mybir.dt.bfloat16
    AF = mybir.ActivationFunctionType

    nc = bacc.Bacc()
    xq_d = nc.declare_dram_parameter("xq", [D, S], F32R, isOutput=False)
    xk_d = nc.declare_dram_parameter("xk", [D, S], F32R, isOutput=False)
    xv_d = nc.declare_dram_parameter("xv", [D, S], BF16, isOutput=False)
    wq_d = nc.declare_dram_parameter("wq", [D, N], F32R, isOutput=False)
    wk_d = nc.declare_dram_parameter("wk", [D, N], F32R, isOutput=False)
    wv_d = nc.declare_dram_parameter("wv", [D, N], BF16, isOutput=False)
    wo_d = nc.declare_dram_parameter("wo", [N, D], BF16, isOutput=False)
    bq_d = nc.declare_dram_parameter("bq", [PAIRS, 128], F32, isOutput=False)
    ones_d = nc.declare_dram_parameter("ones", [128, HPC], BF16, isOutput=False)
    yt_d = nc.dram_tensor("yt", [D, S], F32, kind="ExternalOutput")

    with tile.TileContext(nc) as tc:
        with (
            tc.tile_pool(name="per", bufs=1) as per,
            tc.tile_pool(name="wp", bufs=1) as wp,
            tc.tile_pool(name="xs", bufs=1) as xsp,
            tc.tile_pool(name="ep", bufs=1) as epp,
            tc.tile_pool(name="msc", bufs=1) as msc,
            tc.tile_pool(name="pp", bufs=1, space="PSUM") as pp,
        ):
            # ---- persistent tiles ----
            kh = [per.tile([128, S], F32R, name=f"kh{p}", tag="kh", bufs=PAIRS)
                  for p in range(PAIRS)]
            qh = [per.tile([128, S], F32R, name=f"qh{p}", tag="qh", bufs=PAIRS)
                  for p in range(PAIRS)]
            vs = [per.tile([128, HPC * 65], BF16, name=f"vs{t}", tag="vs", bufs=KC)
                  for t in range(KC)]
            bqt = per.tile([128, PAIRS], F32, name="bqt", tag="bqt", bufs=1)
            for p in range(PAIRS):
                nc.sync.dma_start(out=bqt[:, p:p + 1], in_=bq_d[p, :])

            # ---- weights (tag-shared slots; wo reuses a freed slot later) ----
            wk_s = wp.tile([128, CC, N], F32R, name="wk_s", tag="w2", bufs=2)
            wq_s = wp.tile([128, CC, N], F32R, name="wq_s", tag="w2", bufs=2)
            wv_s = wp.tile([128, CC, N], BF16, name="wv_s", tag="wv", bufs=1)
            nc.sync.dma_start(out=wk_s, in_=wk_d.rearrange("(c p) n -> p c n", p=128))
            nc.scalar.dma_start(out=wv_s, in_=wv_d.rearrange("(c p) n -> p c n", p=128))
            nc.sync.dma_start(out=wq_s, in_=wq_d.rearrange("(c p) n -> p c n", p=128))

            # PSUM pools: "big" 2x[128,1024]f32 (scores + oproj), "sm"
            # 4x[128,512]f32 (phase-1 proj psums, then AV po0/po1).
            # 2*2 + 4*1 = 8 banks.

            # ================= phase 1: projections =================
            # K-proj and Q-proj: out [pair-d 128, seq] per (pair, qblock)
            for w_s, dst, x_d, biased in ((wk_s, kh, xk_d, False),
                                          (wq_s, qh, xq_d, True)):
                for j in range(QB):
                    xt = [xsp.tile([128, N], F32R, name=f"xt{c}", tag="xs", bufs=8)
                          for c in range(CC)]
                    for c in range(CC):
                        eng = nc.sync if c % 2 == 0 else nc.scalar
                        eng.dma_start(
                            out=xt[c],
                            in_=x_d[128 * c:128 * (c + 1), N * j:N * (j + 1)])
                    for p in range(PAIRS):
                        ps = pp.tile([128, N], F32, name="ps", tag="sm", bufs=4)
                        for c in range(CC):
                            nc.tensor.matmul(ps, w_s[:, c, 128 * p:128 * (p + 1)],
                                             xt[c], start=(c == 0),
                                             stop=(c == CC - 1))
                        if biased:
                            nc.vector.tensor_scalar_add(
                                dst[p][:, N * j:N * (j + 1)], ps, bqt[:, p:p + 1])
                        else:
                            nc.vector.tensor_copy(dst[p][:, N * j:N * (j + 1)], ps)
            # V-proj: out [k 128, dv 512] per k-tile; strided into vs + ones col
            for q4 in range(QB):
                xvt = [xsp.tile([128, N], BF16, name=f"xvt{c}", tag="xv", bufs=8)
                       for c in range(CC)]
                for c in range(CC):
                    eng = nc.sync if c % 2 == 0 else nc.scalar
                    eng.dma_start(
                        out=xvt[c],
                        in_=xv_d[128 * c:128 * (c + 1), N * q4:N * (q4 + 1)])
                for t2 in range(4):
                    t = 4 * q4 + t2
                    ps = pp.tile([128, N], F32, name="psv", tag="sm", bufs=4)
                    for c in range(CC):
                        nc.tensor.matmul(ps, xvt[c][:, 128 * t2:128 * (t2 + 1)],
                                         wv_s[:, c, :], start=(c == 0),
                                         stop=(c == CC - 1))
                    v3 = vs[t].rearrange("p (h e) -> p h e", e=65)
                    nc.sync.dma_start(out=v3[:, :, 64:65], in_=ones_d[:, :])
                    nc.vector.tensor_copy(
                        v3[:, :, 0:64], ps.rearrange("p (h e) -> p h e", e=64))

            # wo loaded into a freed w2 slot
            wo_s = wp.tile([128, PAIRS, D], BF16, name="wo_s", tag="w2", bufs=2)
            nc.scalar.dma_start(out=wo_s, in_=wo_d.rearrange("(i p) n -> p i n", p=128))

            # ================= phase 2: attention + out-proj =================
            # Flattened (j, p, g) stream: AV lags scores/exp by LAG groups so
            # PE never waits on ACT; the out-projection of qblock j is
            # interleaved into qblock j+1.
            NG = KC // 2
            stream = [(j, p, g) for j in range(QB) for p in range(PAIRS)
                      for g in range(NG)]
            ctx = {}     # (j, p) -> dict(po0, po1, eA[g], eB[g])
            ots = {}     # j -> [ot tiles]
            oproj_pending = []

            def emit_scores_exp(j, p, g):
                if g == 0:
                    ctx[(j, p)] = {
                        "po0": pp.tile([65, N], F32, name="po0", tag="sm", bufs=4),
                        "po1": pp.tile([65, N], F32, name="po1", tag="sm", bufs=4),
                        "eA": [None] * NG, "eB": [None] * NG,
                    }
                st_ = ctx[(j, p)]
                sA = pp.tile([128, 2 * N], F32, name="sA", tag="big", bufs=2)
                sB = pp.tile([128, 2 * N], F32, name="sB", tag="big", bufs=2)
                for ci in range(2):
                    c = 2 * g + ci
                    nc.tensor.matmul(
                        sA[:, N * ci:N * (ci + 1)],
                        kh[p][0:64, 128 * c:128 * (c + 1)],
                        qh[p][0:64, N * j:N * (j + 1)],
                        start=True, stop=True, tile_position=(0, 0))
                    nc.tensor.matmul(
                        sB[:, N * ci:N * (ci + 1)],
                        kh[p][64:128, 128 * c:128 * (c + 1)],
                        qh[p][64:128, N * j:N * (j + 1)],
                        start=True, stop=True, tile_position=(64, 0))
                eA = epp.tile([128, 2 * N], BF16, name="eA", tag="ep", bufs=4)
                eB = epp.tile([128, 2 * N], BF16, name="eB", tag="ep", bufs=4)
                nc.scalar.activation(eA, sA, AF.Exp, scale=0.125)
                nc.scalar.activation(eB, sB, AF.Exp, scale=0.125)
                st_["eA"][g], st_["eB"][g] = eA, eB

            def emit_av(j, p, g):
                st_ = ctx[(j, p)]
                h0, h1 = 2 * p, 2 * p + 1
                for ci in range(2):
                    c = 2 * g + ci
                    ss, se = (c == 0), (c == KC - 1)
                    nc.tensor.matmul(
                        st_["po0"], vs[c][:, 65 * h0:65 * h0 + 65],
                        st_["eA"][g][:, N * ci:N * (ci + 1)],
                        start=ss, stop=se)
                    nc.tensor.matmul(
                        st_["po1"], vs[c][:, 65 * h1:65 * h1 + 65],
                        st_["eB"][g][:, N * ci:N * (ci + 1)],
                        start=ss, stop=se)

            def emit_norm(j, p):
                st_ = ctx.pop((j, p))
                if j not in ots:
                    ots[j] = [epp.tile([128, N], BF16, name=f"ot{q}", tag="ot",
                                       bufs=8) for q in range(PAIRS)]
                ot = ots[j]
                # Z rows (psum row 64) -> sbuf, reciprocal, broadcast, scale
                zrow = msc.tile([1, 2 * N], F32, name="zrow", tag="zrow", bufs=2)
                nc.vector.tensor_copy(zrow[:, 0:N], st_["po0"][64:65, :])
                nc.vector.tensor_copy(zrow[:, N:2 * N], st_["po1"][64:65, :])
                rz = msc.tile([1, 2 * N], F32, name="rz", tag="rz", bufs=2)
                nc.vector.reciprocal_approx_fast(rz, zrow)
                rbcA = msc.tile([64, N], F32, name="rbcA", tag="rbcA", bufs=2)
                rbcB = msc.tile([64, N], F32, name="rbcB", tag="rbcB", bufs=2)
                nc.gpsimd.partition_broadcast(rbcA, rz[0:1, 0:N])
                nc.gpsimd.partition_broadcast(rbcB, rz[0:1, N:2 * N])
                nc.vector.tensor_mul(ot[p][0:64, :], st_["po0"][0:64, :], rbcA)
                tmp1 = msc.tile([64, N], BF16, name="tmp1", tag="tmp1", bufs=2)
                nc.vector.tensor_mul(tmp1, st_["po1"][0:64, :], rbcB)
                nc.gpsimd.dma_start(out=ot[p][64:128, :], in_=tmp1)
                if p == PAIRS - 1:
                    for e in range(8):
                        oproj_pending.append((j, e))

            def emit_oproj_chunk():
                j2, e = oproj_pending.pop(0)
                ot = ots[j2]
                py = pp.tile([128, N], F32, name="py", tag="big", bufs=2)
                for p2 in range(PAIRS):
                    nc.tensor.matmul(py, wo_s[:, p2, 128 * e:128 * (e + 1)],
                                     ot[p2], start=(p2 == 0), stop=(p2 == PAIRS - 1))
                ys = msc.tile([128, N], F32, name="ys", tag="ys", bufs=2)
                nc.vector.tensor_copy(ys, py)
                nc.sync.dma_start(
                    out=yt_d[128 * e:128 * (e + 1), N * j2:N * (j2 + 1)], in_=ys)
                if e == 7:
                    del ots[j2]

            LAG = 2
            for idx, (j, p, g) in enumerate(stream):
                emit_scores_exp(j, p, g)
                if idx >= LAG:
                    j2, p2, g2 = stream[idx - LAG]
                    emit_av(j2, p2, g2)
                    if g2 == NG - 1:
                        emit_norm(j2, p2)
                if oproj_pending:
                    emit_oproj_chunk()
            for k in range(LAG):
                j2, p2, g2 = stream[len(stream) - LAG + k]
                emit_av(j2, p2, g2)
                if g2 == NG - 1:
                    emit_norm(j2, p2)
            while oproj_pending:
                emit_oproj_chunk()

    nc.compile()
    return nc


def _get_nc():
    if "nc" not in _CACHE:
        _CACHE["nc"] = _build()
    return _CACHE["nc"]


def kernel(q, k, v, w_q, b_q, w_k, b_k, w_v, b_v, w_o, b_o):
    import ml_dtypes

    nc = _get_nc()
    from concourse.bass_utils import run_bass_kernel_spmd

    BF = ml_dtypes.bfloat16
    q = np.asarray(q, dtype=np.float32)
    k = np.asarray(k, dtype=np.float32)
    v = np.asarray(v, dtype=np.float32)
    w_q = np.asarray(w_q, dtype=np.float32)
    w_k = np.asarray(w_k, dtype=np.float32)
    w_v = np.asarray(w_v, dtype=np.float32)
    w_o = np.asarray(w_o, dtype=np.float32)
    b_q = np.asarray(b_q, dtype=np.float32)
    b_v = np.asarray(b_v, dtype=np.float32)
    b_o = np.asarray(b_o, dtype=np.float32)

    xT = {}
    for b in range(B):
        xT[("q", b)] = np.ascontiguousarray(q[b].T)
        xT[("k", b)] = np.ascontiguousarray(k[b].T)
        xT[("v", b)] = np.ascontiguousarray(v[b].T.astype(BF))
    in_maps = []
    for g in range(8):
        b, hg = g // 2, g % 2
        sl = slice(hg * 512, (hg + 1) * 512)
        in_maps.append({
            "xq": xT[("q", b)], "xk": xT[("k", b)], "xv": xT[("v", b)],
            "wq": np.ascontiguousarray(w_q[:, sl]),
            "wk": np.ascontiguousarray(w_k[:, sl]),
            "wv": np.ascontiguousarray(w_v[:, sl].astype(BF)),
            "wo": np.ascontiguousarray(w_o[sl, :].astype(BF)),
            "bq": np.ascontiguousarray(b_q[sl].reshape(PAIRS, 128)),
            "ones": np.ones((128, HPC), dtype=BF),
        })

    res = run_bass_kernel_spmd(nc, in_maps, list(range(8)), trace=False)
    outs = [r["yt"] for r in res.results]

    corr = b_v @ w_o + b_o  # [1024]
    y = np.empty((B, S, D), dtype=np.float32)
    for b in range(B):
        y[b] = outs[2 * b].T + outs[2 * b + 1].T + corr
    return y


# revision 3
# speedup vs baseline: 1.3104x; 1.0703x over previous
"""MultiHeadAttention Trainium2 kernel (8 NeuronCores).

Sharding: 8 cores = 4 batches x 2 head-groups (8 heads each).
Core g: batch b = g//2, head-group hg = g%2 (heads hg*8 .. hg*8+7).

Device program (identical on all cores, SPMD):
  inputs (per core): xq/xk = x[b].T  [1024, 2048] (f32r), xv bf16,
    wq/wk = w[:, hg*512:(hg+1)*512]  [1024, 512] (f32r), wv bf16,
    wo = w_o[hg*512:(hg+1)*512, :]   [512, 1024] (bf16),
    bq = b_q slice reshaped [4, 128] (f32)
  output: yt [1024, 2048] = (partial out).T for this batch/head-group,
    unnormalized by biases (host adds b_v @ w_o + b_o once per batch).

Math identities used (exact in real arithmetic):
  softmax((Q+bq)(K+bk)^T) == softmax((Q+bq) K^T)   [k-constant terms cancel]
  attn @ (V + bv) @ Wo + bo == attn @ V @ Wo + (bv @ Wo + bo)  [rows sum to 1]
  exp without max-subtraction is safe: scores ~ N(0,1), max < ~6.

Dtypes: Q/K path f32r (score exponents are precision-sensitive); V/exp/AV/
out-proj bf16 (post-softmax linear path, errors average out); psum f32.

Matmuls use full K=128 contraction everywhere except QK^T scores (d=64
per head, two heads packed as row-tile pairs at (0,0)/(64,0)).

Layouts: QhT/KhT [128 = head-pair d, 2048 seq] per pair; Vh [128 k-chunk,
8 heads x (64 dv + ones-col)] bf16; ones-col makes the AV matmul also
produce Z = sum_k exp(s) at psum row 64.
"""
import numpy as np

B, S, D = 4, 2048, 1024
HPC, PAIRS, QB, KC, CC = 8, 4, 4, 16, 8  # heads/core, pairs, 512-q-blocks, 128-k-chunks, 128-c-chunks
N = 512

_CACHE = {}


def _build():
    from concourse import bacc
    import concourse.mybir as mybir
    import concourse.tile as tile

    F32 = mybir.dt.float32
    F32R = mybir.dt.float32r
    BF16 = mybir.dt.bfloat16
    AF = mybir.ActivationFunctionType

    nc = bacc.Bacc()
    xq_d = nc.declare_dram_parameter("xq", [D, S], F32R, isOutput=False)
    xk_d = nc.declare_dram_parameter("xk", [D, S], F32R, isOutput=False)
    xv_d = nc.declare_dram_parameter("xv", [D, S], BF16, isOutput=False)
    wq_d = nc.declare_dram_parameter("wq", [D, N], F32R, isOutput=False)
    wk_d = nc.declare_dram_parameter("wk", [D, N], F32R, isOutput=False)
    wv_d = nc.declare_dram_parameter("wv", [D, N], BF16, isOutput=False)
    wo_d = nc.declare_dram_parameter("wo", [N, D], BF16, isOutput=False)
    bq_d = nc.declare_dram_parameter("bq", [PAIRS, 128], F32, isOutput=False)
    ones_d = nc.declare_dram_parameter("ones", [128, HPC], BF16, isOutput=False)
    yt_d = nc.dram_tensor("yt", [D, S], F32, kind="ExternalOutput")

    with tile.TileContext(nc) as tc:
        with (
            tc.tile_pool(name="per", bufs=1) as per,
            tc.tile_pool(name="wp", bufs=1) as wp,
            tc.tile_pool(name="xs", bufs=1) as xsp,
            tc.tile_pool(name="ep", bufs=1) as epp,
            tc.tile_pool(name="msc", bufs=1) as msc,
            tc.tile_pool(name="pp", bufs=1, space="PSUM") as pp,
        ):
            # ---- persistent tiles ----
            kh = [per.tile([128, S], F32R, name=f"kh{p}", tag="kh", bufs=PAIRS)
                  for p in range(PAIRS)]
            qh = [per.tile([128, S], F32R, name=f"qh{p}", tag="qh", bufs=PAIRS)
                  for p in range(PAIRS)]
            vs = [per.tile([128, HPC * 65], BF16, name=f"vs{t}", tag="vs", bufs=KC)
                  for t in range(KC)]
            bqt = per.tile([128, PAIRS], F32, name="bqt", tag="bqt", bufs=1)
            for p in range(PAIRS):
                nc.sync.dma_start(out=bqt[:, p:p + 1], in_=bq_d[p, :])

            # ---- weights (tag-shared slots; wo reuses a freed slot later) ----
            wk_s = wp.tile([128, CC, N], F32R, name="wk_s", tag="w2", bufs=2)
            wq_s = wp.tile([128, CC, N], F32R, name="wq_s", tag="w2", bufs=2)
            wv_s = wp.tile([128, CC, N], BF16, name="wv_s", tag="wv", bufs=1)
            nc.sync.dma_start(out=wk_s, in_=wk_d.rearrange("(c p) n -> p c n", p=128))
            nc.scalar.dma_start(out=wv_s, in_=wv_d.rearrange("(c p) n -> p c n", p=128))
            nc.sync.dma_start(out=wq_s, in_=wq_d.rearrange("(c p) n -> p c n", p=128))

            # PSUM pools: "big" 2x[128,1024]f32 (scores + oproj), "sm"
            # 4x[128,512]f32 (phase-1 proj psums, then AV po0/po1).
            # 2*2 + 4*1 = 8 banks.

            # ================= phase 1: projections =================
            # K-proj and Q-proj: out [pair-d 128, seq] per (pair, qblock)
            for w_s, dst, x_d, biased in ((wk_s, kh, xk_d, False),
                                          (wq_s, qh, xq_d, True)):
                for j in range(QB):
                    xt = [xsp.tile([128, N], F32R, name=f"xt{c}", tag="xs", bufs=8)
                          for c in range(CC)]
                    for c in range(CC):
                        eng = nc.sync if c % 2 == 0 else nc.scalar
                        eng.dma_start(
                            out=xt[c],
                            in_=x_d[128 * c:128 * (c + 1), N * j:N * (j + 1)])
                    for p in range(PAIRS):
                        ps = pp.tile([128, N], F32, name="ps", tag="sm", bufs=4)
                        for c in range(CC):
                            nc.tensor.matmul(ps, w_s[:, c, 128 * p:128 * (p + 1)],
                                             xt[c], start=(c == 0),
                                             stop=(c == CC - 1))
                        if biased:
                            nc.vector.tensor_scalar_add(
                                dst[p][:, N * j:N * (j + 1)], ps, bqt[:, p:p + 1])
                        else:
                            nc.vector.tensor_copy(dst[p][:, N * j:N * (j + 1)], ps)
            # V-proj: out [k 128, dv 512] per k-tile; strided into vs + ones col
            for q4 in range(QB):
                xvt = [xsp.tile([128, N], BF16, name=f"xvt{c}", tag="xv", bufs=8)
                       for c in range(CC)]
                for c in range(CC):
                    eng = nc.sync if c % 2 == 0 else nc.scalar
                    eng.dma_start(
                        out=xvt[c],
                        in_=xv_d[128 * c:128 * (c + 1), N * q4:N * (q4 + 1)])
                for t2 in range(4):
                    t = 4 * q4 + t2
                    ps = pp.tile([128, N], F32, name="psv", tag="sm", bufs=4)
                    for c in range(CC):
                        nc.tensor.matmul(ps, xvt[c][:, 128 * t2:128 * (t2 + 1)],
                                         wv_s[:, c, :], start=(c == 0),
                                         stop=(c == CC - 1))
                    v3 = vs[t].rearrange("p (h e) -> p h e", e=65)
                    nc.sync.dma_start(out=v3[:, :, 64:65], in_=ones_d[:, :])
                    nc.vector.tensor_copy(
                        v3[:, :, 0:64], ps.rearrange("p (h e) -> p h e", e=64))

            # wo loaded into a freed w2 slot
            wo_s = wp.tile([128, PAIRS, D], BF16, name="wo_s", tag="w2", bufs=2)
            nc.scalar.dma_start(out=wo_s, in_=wo_d.rearrange("(i p) n -> p i n", p=128))

            # ================= phase 2: attention + out-proj =================
            # Flattened (j, p, g) stream: AV lags scores/exp by LAG groups so
            # PE never waits on ACT; the out-projection of qblock j is
            # interleaved into qblock j+1.
            NG = KC // 2
            stream = [(j, p, g) for j in range(QB) for p in range(PAIRS)
                      for g in range(NG)]
            ctx = {}     # (j, p) -> dict(po0, po1, eA[g], eB[g])
            ots = {}     # j -> [ot tiles]
            oproj_pending = []

            def emit_scores_exp(j, p, g):
                if g == 0:
                    ctx[(j, p)] = {
                        "po0": pp.tile([65, N], F32, name="po0", tag="sm", bufs=4),
                        "po1": pp.tile([65, N], F32, name="po1", tag="sm", bufs=4),
                        "e": [None] * KC,
                    }
                st_ = ctx[(j, p)]
                # Both heads' scores for one k-chunk share ONE psum tile so
                # the (0,0)/(64,0) pair has identical deps -> the scheduler
                # keeps them adjacent -> row-tile pair co-streams on the PE.
                for ci in range(2):
                    c = 2 * g + ci
                    sc = pp.tile([128, 2 * N], F32, name="sc", tag="big", bufs=2)
                    nc.tensor.matmul(
                        sc[:, 0:N],
                        kh[p][0:64, 128 * c:128 * (c + 1)],
                        qh[p][0:64, N * j:N * (j + 1)],
                        start=True, stop=True, tile_position=(0, 0))
                    nc.tensor.matmul(
                        sc[:, N:2 * N],
                        kh[p][64:128, 128 * c:128 * (c + 1)],
                        qh[p][64:128, N * j:N * (j + 1)],
                        start=True, stop=True, tile_position=(64, 0))
                    ec = epp.tile([128, 2 * N], BF16, name="ec", tag="ep", bufs=4)
                    nc.scalar.activation(ec, sc, AF.Exp, scale=0.125)
                    st_["e"][c] = ec

            def emit_av(j, p, g):
                st_ = ctx[(j, p)]
                h0, h1 = 2 * p, 2 * p + 1
                for ci in range(2):
                    c = 2 * g + ci
                    ss, se = (c == 0), (c == KC - 1)
                    nc.tensor.matmul(
                        st_["po0"], vs[c][:, 65 * h0:65 * h0 + 65],
                        st_["e"][c][:, 0:N],
                        start=ss, stop=se)
                    nc.tensor.matmul(
                        st_["po1"], vs[c][:, 65 * h1:65 * h1 + 65],
                        st_["e"][c][:, N:2 * N],
                        start=ss, stop=se)

            def emit_norm(j, p):
                st_ = ctx.pop((j, p))
                if j not in ots:
                    ots[j] = [epp.tile([128, N], BF16, name=f"ot{q}", tag="ot",
                                       bufs=8) for q in range(PAIRS)]
                ot = ots[j]
                # Z rows (psum row 64) -> sbuf, reciprocal, broadcast, scale
                zrow = msc.tile([1, 2 * N], F32, name="zrow", tag="zrow", bufs=2)
                nc.vector.tensor_copy(zrow[:, 0:N], st_["po0"][64:65, :])
                nc.vector.tensor_copy(zrow[:, N:2 * N], st_["po1"][64:65, :])
                rz = msc.tile([1, 2 * N], F32, name="rz", tag="rz", bufs=2)
                nc.vector.reciprocal_approx_fast(rz, zrow)
                rbcA = msc.tile([64, N], F32, name="rbcA", tag="rbcA", bufs=2)
                rbcB = msc.tile([64, N], F32, name="rbcB", tag="rbcB", bufs=2)
                nc.gpsimd.partition_broadcast(rbcA, rz[0:1, 0:N])
                nc.gpsimd.partition_broadcast(rbcB, rz[0:1, N:2 * N])
                nc.vector.tensor_mul(ot[p][0:64, :], st_["po0"][0:64, :], rbcA)
                tmp1 = msc.tile([64, N], BF16, name="tmp1", tag="tmp1", bufs=2)
                nc.vector.tensor_mul(tmp1, st_["po1"][0:64, :], rbcB)
                nc.gpsimd.dma_start(out=ot[p][64:128, :], in_=tmp1)
                if p == PAIRS - 1:
                    for e in range(8):
                        oproj_pending.append((j, e))

            def emit_oproj_chunk():
                j2, e = oproj_pending.pop(0)
                ot = ots[j2]
                py = pp.tile([128, N], F32, name="py", tag="big", bufs=2)
                for p2 in range(PAIRS):
                    nc.tensor.matmul(py, wo_s[:, p2, 128 * e:128 * (e + 1)],
                                     ot[p2], start=(p2 == 0), stop=(p2 == PAIRS - 1))
                ys = msc.tile([128, N], F32, name="ys", tag="ys", bufs=2)
                nc.vector.tensor_copy(ys, py)
                nc.sync.dma_start(
                    out=yt_d[128 * e:128 * (e + 1), N * j2:N * (j2 + 1)], in_=ys)
                if e == 7:
                    del ots[j2]

            LAG = 2
            for idx, (j, p, g) in enumerate(stream):
                emit_scores_exp(j, p, g)
                if idx >= LAG:
                    j2, p2, g2 = stream[idx - LAG]
                    emit_av(j2, p2, g2)
                    if g2 == NG - 1:
                        emit_norm(j2, p2)
                if oproj_pending:
                    emit_oproj_chunk()
            for k in range(LAG):
                j2, p2, g2 = stream[len(stream) - LAG + k]
                emit_av(j2, p2, g2)
                if g2 == NG - 1:
                    emit_norm(j2, p2)
            while oproj_pending:
                emit_oproj_chunk()

    nc.compile()
    return nc


def _get_nc():
    if "nc" not in _CACHE:
        _CACHE["nc"] = _build()
    return _CACHE["nc"]


def kernel(q, k, v, w_q, b_q, w_k, b_k, w_v, b_v, w_o, b_o):
    import ml_dtypes

    nc = _get_nc()
    from concourse.bass_utils import run_bass_kernel_spmd

    BF = ml_dtypes.bfloat16
    q = np.asarray(q, dtype=np.float32)
    k = np.asarray(k, dtype=np.float32)
    v = np.asarray(v, dtype=np.float32)
    w_q = np.asarray(w_q, dtype=np.float32)
    w_k = np.asarray(w_k, dtype=np.float32)
    w_v = np.asarray(w_v, dtype=np.float32)
    w_o = np.asarray(w_o, dtype=np.float32)
    b_q = np.asarray(b_q, dtype=np.float32)
    b_v = np.asarray(b_v, dtype=np.float32)
    b_o = np.asarray(b_o, dtype=np.float32)

    xT = {}
    for b in range(B):
        xT[("q", b)] = np.ascontiguousarray(q[b].T)
        xT[("k", b)] = np.ascontiguousarray(k[b].T)
        xT[("v", b)] = np.ascontiguousarray(v[b].T.astype(BF))
    in_maps = []
    for g in range(8):
        b, hg = g // 2, g % 2
        sl = slice(hg * 512, (hg + 1) * 512)
        in_maps.append({
            "xq": xT[("q", b)], "xk": xT[("k", b)], "xv": xT[("v", b)],
            "wq": np.ascontiguousarray(w_q[:, sl]),
            "wk": np.ascontiguousarray(w_k[:, sl]),
            "wv": np.ascontiguousarray(w_v[:, sl].astype(BF)),
            "wo": np.ascontiguousarray(w_o[sl, :].astype(BF)),
            "bq": np.ascontiguousarray(b_q[sl].reshape(PAIRS, 128)),
            "ones": np.ones((128, HPC), dtype=BF),
        })

    res = run_bass_kernel_spmd(nc, in_maps, list(range(8)), trace=False)
    outs = [r["yt"] for r in res.results]

    corr = b_v @ w_o + b_o  # [1024]
    y = np.empty((B, S, D), dtype=np.float32)
    for b in range(B):
        y[b] = outs[2 * b].T + outs[2 * b + 1].T + corr
    return y


# revision 4
# speedup vs baseline: 1.4177x; 1.0819x over previous
"""MultiHeadAttention Trainium2 kernel (8 NeuronCores).

Sharding: 8 cores = 4 batches x 2 head-groups (8 heads each).
Core g: batch b = g//2, head-group hg = g%2 (heads hg*8 .. hg*8+7).

Device program (identical on all cores, SPMD):
  inputs (per core): xq/xk = x[b].T  [1024, 2048] (f32r), xv bf16,
    wq/wk = w[:, hg*512:(hg+1)*512]  [1024, 512] (f32r), wv bf16,
    wo = w_o[hg*512:(hg+1)*512, :]   [512, 1024] (bf16),
    bq = b_q slice reshaped [4, 128] (f32)
  output: yt [1024, 2048] = (partial out).T for this batch/head-group,
    unnormalized by biases (host adds b_v @ w_o + b_o once per batch).

Math identities used (exact in real arithmetic):
  softmax((Q+bq)(K+bk)^T) == softmax((Q+bq) K^T)   [k-constant terms cancel]
  attn @ (V + bv) @ Wo + bo == attn @ V @ Wo + (bv @ Wo + bo)  [rows sum to 1]
  exp without max-subtraction is safe: scores ~ N(0,1), max < ~6.

Dtypes: Q/K path f32r (score exponents are precision-sensitive); V/exp/AV/
out-proj bf16 (post-softmax linear path, errors average out); psum f32.

Matmuls use full K=128 contraction everywhere except QK^T scores (d=64
per head, two heads packed as row-tile pairs at (0,0)/(64,0)).

Layouts: QhT/KhT [128 = head-pair d, 2048 seq] per pair; Vh [128 k-chunk,
8 heads x (64 dv + ones-col)] bf16; ones-col makes the AV matmul also
produce Z = sum_k exp(s) at psum row 64.
"""
import numpy as np

B, S, D = 4, 2048, 1024
HPC, PAIRS, QB, KC, CC = 8, 4, 4, 16, 8  # heads/core, pairs, 512-q-blocks, 128-k-chunks, 128-c-chunks
N = 512

_CACHE = {}


def _build():
    from concourse import bacc
    import concourse.mybir as mybir
    import concourse.tile as tile

    F32 = mybir.dt.float32
    F32R = mybir.dt.float32r
    BF16 = mybir.dt.bfloat16
    AF = mybir.ActivationFunctionType

    nc = bacc.Bacc()
    xq_d = nc.declare_dram_parameter("xq", [D, S], BF16, isOutput=False)
    xk_d = nc.declare_dram_parameter("xk", [D, S], BF16, isOutput=False)
    xv_d = nc.declare_dram_parameter("xv", [D, S], BF16, isOutput=False)
    wq_d = nc.declare_dram_parameter("wq", [D, N], BF16, isOutput=False)
    wk_d = nc.declare_dram_parameter("wk", [D, N], BF16, isOutput=False)
    wv_d = nc.declare_dram_parameter("wv", [D, N], BF16, isOutput=False)
    wo_d = nc.declare_dram_parameter("wo", [N, D], BF16, isOutput=False)
    bq_d = nc.declare_dram_parameter("bq", [PAIRS, 128], F32, isOutput=False)
    ones_d = nc.declare_dram_parameter("ones", [128, HPC], BF16, isOutput=False)
    yt_d = nc.dram_tensor("yt", [D, S], F32, kind="ExternalOutput")

    with tile.TileContext(nc) as tc:
        with (
            tc.tile_pool(name="per", bufs=1) as per,
            tc.tile_pool(name="wp", bufs=1) as wp,
            tc.tile_pool(name="xs", bufs=1) as xsp,
            tc.tile_pool(name="ep", bufs=1) as epp,
            tc.tile_pool(name="msc", bufs=1) as msc,
            tc.tile_pool(name="pp", bufs=1, space="PSUM") as pp,
        ):
            # ---- persistent tiles ----
            kh = [per.tile([128, S], F32R, name=f"kh{p}", tag="kh", bufs=PAIRS)
                  for p in range(PAIRS)]
            qh = [per.tile([128, S], F32R, name=f"qh{p}", tag="qh", bufs=PAIRS)
                  for p in range(PAIRS)]
            vs = [per.tile([128, HPC * 65], BF16, name=f"vs{t}", tag="vs", bufs=KC)
                  for t in range(KC)]
            bqt = per.tile([128, PAIRS], F32, name="bqt", tag="bqt", bufs=1)
            for p in range(PAIRS):
                nc.scalar.dma_start(out=bqt[:, p:p + 1], in_=bq_d[p, :])

            # ---- weights (tag-shared slots; wo reuses a freed slot later) ----
            wk_s = wp.tile([128, CC, N], BF16, name="wk_s", tag="w2", bufs=2)
            wq_s = wp.tile([128, CC, N], BF16, name="wq_s", tag="w2", bufs=2)
            wv_s = wp.tile([128, CC, N], BF16, name="wv_s", tag="wv", bufs=1)
            nc.scalar.dma_start(out=wk_s, in_=wk_d.rearrange("(c p) n -> p c n", p=128))
            nc.scalar.dma_start(out=wv_s, in_=wv_d.rearrange("(c p) n -> p c n", p=128))
            nc.scalar.dma_start(out=wq_s, in_=wq_d.rearrange("(c p) n -> p c n", p=128))

            # PSUM pools: "big" 2x[128,1024]f32 (scores + oproj), "sm"
            # 4x[128,512]f32 (phase-1 proj psums, then AV po0/po1).
            # 2*2 + 4*1 = 8 banks.

            # ================= phase 1: projections =================
            # K-proj and Q-proj: out [pair-d 128, seq] per (pair, qblock)
            for w_s, dst, x_d, biased in ((wk_s, kh, xk_d, False),
                                          (wq_s, qh, xq_d, True)):
                for j in range(QB):
                    xt = [xsp.tile([128, N], BF16, name=f"xt{c}", tag="xs", bufs=8)
                          for c in range(CC)]
                    for c in range(CC):
                        eng = nc.sync if c % 2 == 0 else nc.gpsimd
                        eng.dma_start(
                            out=xt[c],
                            in_=x_d[128 * c:128 * (c + 1), N * j:N * (j + 1)])
                    for p in range(PAIRS):
                        ps = pp.tile([128, N], F32, name="ps", tag="sm", bufs=4)
                        for c in range(CC):
                            nc.tensor.matmul(ps, w_s[:, c, 128 * p:128 * (p + 1)],
                                             xt[c], start=(c == 0),
                                             stop=(c == CC - 1))
                        if biased:
                            nc.vector.tensor_scalar_add(
                                dst[p][:, N * j:N * (j + 1)], ps, bqt[:, p:p + 1])
                        else:
                            nc.vector.tensor_copy(dst[p][:, N * j:N * (j + 1)], ps)
            # V-proj: out [k 128, dv 512] per k-tile; strided into vs + ones col
            for q4 in range(QB):
                xvt = [xsp.tile([128, N], BF16, name=f"xvt{c}", tag="xv", bufs=8)
                       for c in range(CC)]
                for c in range(CC):
                    eng = nc.sync if c % 2 == 0 else nc.gpsimd
                    eng.dma_start(
                        out=xvt[c],
                        in_=xv_d[128 * c:128 * (c + 1), N * q4:N * (q4 + 1)])
                for t2 in range(4):
                    t = 4 * q4 + t2
                    ps = pp.tile([128, N], F32, name="psv", tag="sm", bufs=4)
                    for c in range(CC):
                        nc.tensor.matmul(ps, xvt[c][:, 128 * t2:128 * (t2 + 1)],
                                         wv_s[:, c, :], start=(c == 0),
                                         stop=(c == CC - 1))
                    v3 = vs[t].rearrange("p (h e) -> p h e", e=65)
                    nc.sync.dma_start(out=v3[:, :, 64:65], in_=ones_d[:, :])
                    nc.vector.tensor_copy(
                        v3[:, :, 0:64], ps.rearrange("p (h e) -> p h e", e=64))

            # wo loaded into a freed w2 slot
            wo_s = wp.tile([128, PAIRS, D], BF16, name="wo_s", tag="w2", bufs=2)
            nc.scalar.dma_start(out=wo_s, in_=wo_d.rearrange("(i p) n -> p i n", p=128))

            # ================= phase 2: attention + out-proj =================
            # Flattened (j, p, g) stream: AV lags scores/exp by LAG groups so
            # PE never waits on ACT; the out-projection of qblock j is
            # interleaved into qblock j+1.
            NG = KC // 2
            stream = [(j, p, g) for j in range(QB) for p in range(PAIRS)
                      for g in range(NG)]
            ctx = {}     # (j, p) -> dict(po0, po1, eA[g], eB[g])
            ots = {}     # j -> [ot tiles]
            oproj_pending = []

            def emit_scores_exp(j, p, g):
                if g == 0:
                    ctx[(j, p)] = {
                        "po0": pp.tile([65, N], F32, name="po0", tag="sm", bufs=4),
                        "po1": pp.tile([65, N], F32, name="po1", tag="sm", bufs=4),
                        "e": [None] * KC,
                    }
                st_ = ctx[(j, p)]
                # Both heads' scores for one k-chunk share ONE psum tile so
                # the (0,0)/(64,0) pair has identical deps -> the scheduler
                # keeps them adjacent -> row-tile pair co-streams on the PE.
                for ci in range(2):
                    c = 2 * g + ci
                    sc = pp.tile([128, 2 * N], F32, name="sc", tag="big", bufs=2)
                    nc.tensor.matmul(
                        sc[:, 0:N],
                        kh[p][0:64, 128 * c:128 * (c + 1)],
                        qh[p][0:64, N * j:N * (j + 1)],
                        start=True, stop=True, tile_position=(0, 0))
                    nc.tensor.matmul(
                        sc[:, N:2 * N],
                        kh[p][64:128, 128 * c:128 * (c + 1)],
                        qh[p][64:128, N * j:N * (j + 1)],
                        start=True, stop=True, tile_position=(64, 0))
                    ec = epp.tile([128, 2 * N], BF16, name="ec", tag="ep", bufs=16)
                    nc.scalar.activation(ec, sc, AF.Exp, scale=0.125)
                    st_["e"][c] = ec

            def emit_av(j, p, g):
                st_ = ctx[(j, p)]
                h0, h1 = 2 * p, 2 * p + 1
                for ci in range(2):
                    c = 2 * g + ci
                    ss, se = (c == 0), (c == KC - 1)
                    nc.tensor.matmul(
                        st_["po0"], vs[c][:, 65 * h0:65 * h0 + 65],
                        st_["e"][c][:, 0:N],
                        start=ss, stop=se)
                    nc.tensor.matmul(
                        st_["po1"], vs[c][:, 65 * h1:65 * h1 + 65],
                        st_["e"][c][:, N:2 * N],
                        start=ss, stop=se)

            def emit_norm(j, p):
                st_ = ctx.pop((j, p))
                if j not in ots:
                    ots[j] = [epp.tile([128, N], BF16, name=f"ot{q}", tag="ot",
                                       bufs=8) for q in range(PAIRS)]
                ot = ots[j]
                # Z rows (psum row 64) -> sbuf, reciprocal, broadcast, scale
                zrow = msc.tile([1, 2 * N], F32, name="zrow", tag="zrow", bufs=2)
                nc.vector.tensor_copy(zrow[:, 0:N], st_["po0"][64:65, :])
                nc.vector.tensor_copy(zrow[:, N:2 * N], st_["po1"][64:65, :])
                rz = msc.tile([1, 2 * N], F32, name="rz", tag="rz", bufs=2)
                nc.vector.reciprocal_approx_fast(rz, zrow)
                rbcA = msc.tile([64, N], F32, name="rbcA", tag="rbcA", bufs=2)
                rbcB = msc.tile([64, N], F32, name="rbcB", tag="rbcB", bufs=2)
                nc.gpsimd.partition_broadcast(rbcA, rz[0:1, 0:N])
                nc.gpsimd.partition_broadcast(rbcB, rz[0:1, N:2 * N])
                nc.vector.tensor_mul(ot[p][0:64, :], st_["po0"][0:64, :], rbcA)
                tmp1 = msc.tile([64, N], BF16, name="tmp1", tag="tmp1", bufs=2)
                nc.vector.tensor_mul(tmp1, st_["po1"][0:64, :], rbcB)
                nc.gpsimd.dma_start(out=ot[p][64:128, :], in_=tmp1)
                if p == PAIRS - 1:
                    for e in range(8):
                        oproj_pending.append((j, e))

            def emit_oproj_chunk():
                j2, e = oproj_pending.pop(0)
                ot = ots[j2]
                py = pp.tile([128, N], F32, name="py", tag="sm", bufs=4)
                for p2 in range(PAIRS):
                    nc.tensor.matmul(py, wo_s[:, p2, 128 * e:128 * (e + 1)],
                                     ot[p2], start=(p2 == 0), stop=(p2 == PAIRS - 1))
                ys = msc.tile([128, N], F32, name="ys", tag="ys", bufs=2)
                nc.vector.tensor_copy(ys, py)
                nc.sync.dma_start(
                    out=yt_d[128 * e:128 * (e + 1), N * j2:N * (j2 + 1)], in_=ys)
                if e == 7:
                    del ots[j2]

            LAG = 8
            for idx, (j, p, g) in enumerate(stream):
                emit_scores_exp(j, p, g)
                if idx >= LAG:
                    j2, p2, g2 = stream[idx - LAG]
                    emit_av(j2, p2, g2)
                    if g2 == NG - 1:
                        emit_norm(j2, p2)
                if oproj_pending:
                    emit_oproj_chunk()
            for k in range(LAG):
                j2, p2, g2 = stream[len(stream) - LAG + k]
                emit_av(j2, p2, g2)
                if g2 == NG - 1:
                    emit_norm(j2, p2)
            while oproj_pending:
                emit_oproj_chunk()

    nc.compile()
    return nc


def _get_nc():
    if "nc" not in _CACHE:
        _CACHE["nc"] = _build()
    return _CACHE["nc"]


def kernel(q, k, v, w_q, b_q, w_k, b_k, w_v, b_v, w_o, b_o):
    import ml_dtypes

    nc = _get_nc()
    from concourse.bass_utils import run_bass_kernel_spmd

    BF = ml_dtypes.bfloat16
    q = np.asarray(q, dtype=np.float32)
    k = np.asarray(k, dtype=np.float32)
    v = np.asarray(v, dtype=np.float32)
    w_q = np.asarray(w_q, dtype=np.float32)
    w_k = np.asarray(w_k, dtype=np.float32)
    w_v = np.asarray(w_v, dtype=np.float32)
    w_o = np.asarray(w_o, dtype=np.float32)
    b_q = np.asarray(b_q, dtype=np.float32)
    b_v = np.asarray(b_v, dtype=np.float32)
    b_o = np.asarray(b_o, dtype=np.float32)

    xT = {}
    for b in range(B):
        xT[("q", b)] = np.ascontiguousarray(q[b].T.astype(BF))
        xT[("k", b)] = np.ascontiguousarray(k[b].T.astype(BF))
        xT[("v", b)] = np.ascontiguousarray(v[b].T.astype(BF))
    in_maps = []
    for g in range(8):
        b, hg = g // 2, g % 2
        sl = slice(hg * 512, (hg + 1) * 512)
        in_maps.append({
            "xq": xT[("q", b)], "xk": xT[("k", b)], "xv": xT[("v", b)],
            "wq": np.ascontiguousarray(w_q[:, sl].astype(BF)),
            "wk": np.ascontiguousarray(w_k[:, sl].astype(BF)),
            "wv": np.ascontiguousarray(w_v[:, sl].astype(BF)),
            "wo": np.ascontiguousarray(w_o[sl, :].astype(BF)),
            "bq": np.ascontiguousarray(b_q[sl].reshape(PAIRS, 128)),
            "ones": np.ones((128, HPC), dtype=BF),
        })

    res = run_bass_kernel_spmd(nc, in_maps, list(range(8)), trace=False)
    outs = [r["yt"] for r in res.results]

    corr = b_v @ w_o + b_o  # [1024]
    y = np.empty((B, S, D), dtype=np.float32)
    for b in range(B):
        y[b] = outs[2 * b].T + outs[2 * b + 1].T + corr
    return y


# revision 6
# speedup vs baseline: 1.5712x; 1.1083x over previous
"""MultiHeadAttention Trainium2 kernel (8 NeuronCores).

Sharding: 8 cores = 4 batches x 2 head-groups (8 heads each).
Core g: batch b = g//2, head-group hg = g%2 (heads hg*8 .. hg*8+7).

Device program (identical on all cores, SPMD):
  inputs (per core): xq/xk = x[b].T  [1024, 2048] (f32r), xv bf16,
    wq/wk = w[:, hg*512:(hg+1)*512]  [1024, 512] (f32r), wv bf16,
    wo = w_o[hg*512:(hg+1)*512, :]   [512, 1024] (bf16),
    bq = b_q slice reshaped [4, 128] (f32)
  output: yt [1024, 2048] = (partial out).T for this batch/head-group,
    unnormalized by biases (host adds b_v @ w_o + b_o once per batch).

Math identities used (exact in real arithmetic):
  softmax((Q+bq)(K+bk)^T) == softmax((Q+bq) K^T)   [k-constant terms cancel]
  attn @ (V + bv) @ Wo + bo == attn @ V @ Wo + (bv @ Wo + bo)  [rows sum to 1]
  exp without max-subtraction is safe: scores ~ N(0,1), max < ~6.

Dtypes: Q/K path f32r (score exponents are precision-sensitive); V/exp/AV/
out-proj bf16 (post-softmax linear path, errors average out); psum f32.

Matmuls use full K=128 contraction everywhere except QK^T scores (d=64
per head, two heads packed as row-tile pairs at (0,0)/(64,0)).

Layouts: QhT/KhT [128 = head-pair d, 2048 seq] per pair; Vh [128 k-chunk,
8 heads x (64 dv + ones-col)] bf16; ones-col makes the AV matmul also
produce Z = sum_k exp(s) at psum row 64.
"""
import numpy as np

B, S, D = 4, 2048, 1024
HPC, PAIRS, QB, KC, CC = 8, 4, 4, 16, 8  # heads/core, pairs, 512-q-blocks, 128-k-chunks, 128-c-chunks
N = 512

_CACHE = {}


def _build():
    from concourse import bacc
    import concourse.mybir as mybir
    import concourse.tile as tile

    F32 = mybir.dt.float32
    F32R = mybir.dt.float32r
    BF16 = mybir.dt.bfloat16
    AF = mybir.ActivationFunctionType

    nc = bacc.Bacc()
    xq_d = nc.declare_dram_parameter("xq", [D, S], BF16, isOutput=False)
    xk_d = nc.declare_dram_parameter("xk", [D, S], BF16, isOutput=False)
    xv_d = nc.declare_dram_parameter("xv", [D, S], BF16, isOutput=False)
    wq_d = nc.declare_dram_parameter("wq", [D, N], BF16, isOutput=False)
    wk_d = nc.declare_dram_parameter("wk", [D, N], BF16, isOutput=False)
    wv_d = nc.declare_dram_parameter("wv", [D, N], BF16, isOutput=False)
    wo_d = nc.declare_dram_parameter("wo", [N, D], BF16, isOutput=False)
    bq_d = nc.declare_dram_parameter("bq", [PAIRS, 128], F32, isOutput=False)
    ones_d = nc.declare_dram_parameter("ones", [128, HPC], BF16, isOutput=False)
    yt_d = nc.dram_tensor("yt", [D, S], F32, kind="ExternalOutput")

    with tile.TileContext(nc) as tc:
        with (
            tc.tile_pool(name="per", bufs=1) as per,
            tc.tile_pool(name="wp", bufs=1) as wp,
            tc.tile_pool(name="xs", bufs=1) as xsp,
            tc.tile_pool(name="ep", bufs=1) as epp,
            tc.tile_pool(name="msc", bufs=1) as msc,
            tc.tile_pool(name="pp", bufs=1, space="PSUM") as pp,
        ):
            # ---- persistent tiles ----
            kh = [per.tile([128, S], F32R, name=f"kh{p}", tag="kh", bufs=PAIRS)
                  for p in range(PAIRS)]
            qh = [per.tile([128, S], F32R, name=f"qh{p}", tag="qh", bufs=PAIRS)
                  for p in range(PAIRS)]
            vs = [per.tile([128, HPC * 65], BF16, name=f"vs{t}", tag="vs", bufs=KC)
                  for t in range(KC)]
            # ---- weights (tag-shared slots; wo reuses a freed slot later) ----
            wk_s = wp.tile([128, CC, N], BF16, name="wk_s", tag="w2", bufs=2)
            wq_s = wp.tile([128, CC, N], BF16, name="wq_s", tag="w2", bufs=2)
            wv_s = wp.tile([128, CC, N], BF16, name="wv_s", tag="wv", bufs=1)
            nc.scalar.dma_start(out=wk_s, in_=wk_d.rearrange("(c p) n -> p c n", p=128))
            nc.scalar.dma_start(out=wv_s, in_=wv_d.rearrange("(c p) n -> p c n", p=128))
            nc.scalar.dma_start(out=wq_s, in_=wq_d.rearrange("(c p) n -> p c n", p=128))
            bqt = per.tile([128, PAIRS], F32, name="bqt", tag="bqt", bufs=1)
            for p in range(PAIRS):
                nc.scalar.dma_start(out=bqt[:, p:p + 1], in_=bq_d[p, :])

            # PSUM pools: "big" 2x[128,1024]f32 (scores + oproj), "sm"
            # 4x[128,512]f32 (phase-1 proj psums, then AV po0/po1).
            # 2*2 + 4*1 = 8 banks.

            # ================= phase 1: projections =================
            # K-proj and Q-proj: out [pair-d 128, seq] per (pair, qblock)
            for w_s, dst, x_d, biased in ((wk_s, kh, xk_d, False),
                                          (wq_s, qh, xq_d, True)):
                for j in range(QB):
                    xt = [xsp.tile([128, N], BF16, name=f"xt{c}", tag="xs", bufs=16)
                          for c in range(CC)]
                    for c in range(CC):
                        eng = nc.sync if c % 2 == 0 else nc.gpsimd
                        eng.dma_start(
                            out=xt[c],
                            in_=x_d[128 * c:128 * (c + 1), N * j:N * (j + 1)])
                    for p in range(PAIRS):
                        ps = pp.tile([128, N], F32, name="ps", tag="sm", bufs=4)
                        for c in range(CC):
                            nc.tensor.matmul(ps, w_s[:, c, 128 * p:128 * (p + 1)],
                                             xt[c], start=(c == 0),
                                             stop=(c == CC - 1))
                        if biased:
                            nc.vector.tensor_scalar_add(
                                dst[p][:, N * j:N * (j + 1)], ps, bqt[:, p:p + 1])
                        else:
                            nc.vector.tensor_copy(dst[p][:, N * j:N * (j + 1)], ps)
            # V-proj: out [k 128, dv 512] per k-tile; strided into vs + ones col
            for q4 in range(QB):
                xvt = [xsp.tile([128, N], BF16, name=f"xvt{c}", tag="xv", bufs=16)
                       for c in range(CC)]
                for c in range(CC):
                    eng = nc.sync if c % 2 == 0 else nc.gpsimd
                    eng.dma_start(
                        out=xvt[c],
                        in_=xv_d[128 * c:128 * (c + 1), N * q4:N * (q4 + 1)])
                for t2 in range(4):
                    t = 4 * q4 + t2
                    ps = pp.tile([128, N], F32, name="psv", tag="sm", bufs=4)
                    for c in range(CC):
                        nc.tensor.matmul(ps, xvt[c][:, 128 * t2:128 * (t2 + 1)],
                                         wv_s[:, c, :], start=(c == 0),
                                         stop=(c == CC - 1))
                    v3 = vs[t].rearrange("p (h e) -> p h e", e=65)
                    nc.scalar.dma_start(out=v3[:, :, 64:65], in_=ones_d[:, :])
                    nc.vector.tensor_copy(
                        v3[:, :, 0:64], ps.rearrange("p (h e) -> p h e", e=64))

            # wo loaded into a freed w2 slot
            wo_s = wp.tile([128, PAIRS, D], BF16, name="wo_s", tag="w2", bufs=2)
            nc.scalar.dma_start(out=wo_s, in_=wo_d.rearrange("(i p) n -> p i n", p=128))

            # ================= phase 2: attention + out-proj =================
            # Flattened (j, p, g) stream: AV lags scores/exp by LAG groups so
            # PE never waits on ACT; the out-projection of qblock j is
            # interleaved into qblock j+1.
            NG = KC // 2
            stream = [(j, p, g) for j in range(QB) for p in range(PAIRS)
                      for g in range(NG)]
            ctx = {}     # (j, p) -> dict(po0, po1, eA[g], eB[g])
            ots = {}     # j -> [ot tiles]
            oproj_pending = []

            def emit_scores_exp(j, p, g):
                if g == 0:
                    ctx[(j, p)] = {
                        "po0": pp.tile([65, N], F32, name="po0", tag="sm", bufs=4),
                        "po1": pp.tile([65, N], F32, name="po1", tag="sm", bufs=4),
                        "e": [None] * KC,
                    }
                st_ = ctx[(j, p)]
                # Both heads' scores for one k-chunk share ONE psum tile so
                # the (0,0)/(64,0) pair has identical deps -> the scheduler
                # keeps them adjacent -> row-tile pair co-streams on the PE.
                for ci in range(2):
                    c = 2 * g + ci
                    sc = pp.tile([128, 2 * N], F32, name="sc", tag="big", bufs=2)
                    nc.tensor.matmul(
                        sc[:, 0:N],
                        kh[p][0:64, 128 * c:128 * (c + 1)],
                        qh[p][0:64, N * j:N * (j + 1)],
                        start=True, stop=True, tile_position=(0, 0))
                    nc.tensor.matmul(
                        sc[:, N:2 * N],
                        kh[p][64:128, 128 * c:128 * (c + 1)],
                        qh[p][64:128, N * j:N * (j + 1)],
                        start=True, stop=True, tile_position=(64, 0))
                    ec = epp.tile([128, 2 * N], BF16, name="ec", tag="ep", bufs=16)
                    nc.scalar.activation(ec, sc, AF.Exp, scale=0.125)
                    st_["e"][c] = ec

            def emit_av(j, p, g):
                st_ = ctx[(j, p)]
                h0, h1 = 2 * p, 2 * p + 1
                for ci in range(2):
                    c = 2 * g + ci
                    ss, se = (c == 0), (c == KC - 1)
                    nc.tensor.matmul(
                        st_["po0"], vs[c][:, 65 * h0:65 * h0 + 65],
                        st_["e"][c][:, 0:N],
                        start=ss, stop=se)
                    nc.tensor.matmul(
                        st_["po1"], vs[c][:, 65 * h1:65 * h1 + 65],
                        st_["e"][c][:, N:2 * N],
                        start=ss, stop=se)

            def emit_norm(j, p):
                st_ = ctx.pop((j, p))
                if j not in ots:
                    ots[j] = [epp.tile([128, N], BF16, name=f"ot{q}", tag="ot",
                                       bufs=8) for q in range(PAIRS)]
                ot = ots[j]
                # Z rows (psum row 64) -> sbuf, reciprocal, broadcast, scale
                zrow = msc.tile([1, 2 * N], F32, name="zrow", tag="zrow", bufs=2)
                nc.vector.tensor_copy(zrow[:, 0:N], st_["po0"][64:65, :])
                nc.vector.tensor_copy(zrow[:, N:2 * N], st_["po1"][64:65, :])
                rz = msc.tile([1, 2 * N], F32, name="rz", tag="rz", bufs=2)
                nc.vector.reciprocal_approx_fast(rz, zrow)
                rbcA = msc.tile([64, N], F32, name="rbcA", tag="rbcA", bufs=2)
                rbcB = msc.tile([64, N], F32, name="rbcB", tag="rbcB", bufs=2)
                nc.gpsimd.partition_broadcast(rbcA, rz[0:1, 0:N])
                nc.gpsimd.partition_broadcast(rbcB, rz[0:1, N:2 * N])
                nc.vector.tensor_mul(ot[p][0:64, :], st_["po0"][0:64, :], rbcA)
                tmp1 = msc.tile([64, N], BF16, name="tmp1", tag="tmp1", bufs=2)
                nc.vector.tensor_mul(tmp1, st_["po1"][0:64, :], rbcB)
                nc.gpsimd.dma_start(out=ot[p][64:128, :], in_=tmp1)
                if p == PAIRS - 1:
                    for e in range(8):
                        oproj_pending.append((j, e))

            def emit_oproj_chunk():
                j2, e = oproj_pending.pop(0)
                ot = ots[j2]
                py = pp.tile([128, N], F32, name="py", tag="sm", bufs=4)
                for p2 in range(PAIRS):
                    nc.tensor.matmul(py, wo_s[:, p2, 128 * e:128 * (e + 1)],
                                     ot[p2], start=(p2 == 0), stop=(p2 == PAIRS - 1))
                ys = msc.tile([128, N], F32, name="ys", tag="ys", bufs=2)
                nc.vector.tensor_copy(ys, py)
                nc.sync.dma_start(
                    out=yt_d[128 * e:128 * (e + 1), N * j2:N * (j2 + 1)], in_=ys)
                if e == 7:
                    del ots[j2]

            # LAG: deep early (scores/exp run ahead while phase-1 PE work
            # drains), shallow later (short pure-PE tail after last exp).
            av_done = 0

            def drain_av(upto):
                nonlocal av_done
                while av_done < upto:
                    j2, p2, g2 = stream[av_done]
                    emit_av(j2, p2, g2)
                    if g2 == NG - 1:
                        emit_norm(j2, p2)
                    av_done += 1

            for idx, (j, p, g) in enumerate(stream):
                emit_scores_exp(j, p, g)
                lag = 8 if idx < 64 else 3
                drain_av(idx + 1 - lag)
                if oproj_pending:
                    emit_oproj_chunk()
            drain_av(len(stream))
            while oproj_pending:
                emit_oproj_chunk()

    nc.compile()
    return nc


def _get_nc():
    if "nc" not in _CACHE:
        _CACHE["nc"] = _build()
    return _CACHE["nc"]


def kernel(q, k, v, w_q, b_q, w_k, b_k, w_v, b_v, w_o, b_o):
    import ml_dtypes

    nc = _get_nc()
    from concourse.bass_utils import run_bass_kernel_spmd

    BF = ml_dtypes.bfloat16
    q = np.asarray(q, dtype=np.float32)
    k = np.asarray(k, dtype=np.float32)
    v = np.asarray(v, dtype=np.float32)
    w_q = np.asarray(w_q, dtype=np.float32)
    w_k = np.asarray(w_k, dtype=np.float32)
    w_v = np.asarray(w_v, dtype=np.float32)
    w_o = np.asarray(w_o, dtype=np.float32)
    b_q = np.asarray(b_q, dtype=np.float32)
    b_v = np.asarray(b_v, dtype=np.float32)
    b_o = np.asarray(b_o, dtype=np.float32)

    xT = {}
    for b in range(B):
        xT[("q", b)] = np.ascontiguousarray(q[b].T.astype(BF))
        xT[("k", b)] = np.ascontiguousarray(k[b].T.astype(BF))
        xT[("v", b)] = np.ascontiguousarray(v[b].T.astype(BF))
    in_maps = []
    for g in range(8):
        b, hg = g // 2, g % 2
        sl = slice(hg * 512, (hg + 1) * 512)
        in_maps.append({
            "xq": xT[("q", b)], "xk": xT[("k", b)], "xv": xT[("v", b)],
            "wq": np.ascontiguousarray(w_q[:, sl].astype(BF)),
            "wk": np.ascontiguousarray(w_k[:, sl].astype(BF)),
            "wv": np.ascontiguousarray(w_v[:, sl].astype(BF)),
            "wo": np.ascontiguousarray(w_o[sl, :].astype(BF)),
            "bq": np.ascontiguousarray(b_q[sl].reshape(PAIRS, 128)),
            "ones": np.ones((128, HPC), dtype=BF),
        })

    res = run_bass_kernel_spmd(nc, in_maps, list(range(8)), trace=False)
    outs = [r["yt"] for r in res.results]

    corr = b_v @ w_o + b_o  # [1024]
    y = np.empty((B, S, D), dtype=np.float32)
    for b in range(B):
        y[b] = outs[2 * b].T + outs[2 * b + 1].T + corr
    return y
